# revision 1
# baseline (speedup 1.0000x reference)
"""Fused 2D-RoPE multi-head attention block for Trainium2, SPMD over 8 NeuronCores.

Problem: x[2,4,24,24,1024] -> qkv proj -> 16-head attention with 2-axis RoPE
-> out proj.  Data-parallel: the fused (b t) dim has 8 sequences; one
sequence (S=576 tokens, D=1024) per core.

Device-side layout choices (everything picked so no on-device transposes are
needed):
  - x is fed pre-transposed per core: xT [D, S].
  - q,k are produced in [e, s] layout (head-dim on partitions) by using the
    (host-pre-transposed) weight as the stationary operand.
  - Within each head, q/k weight rows are host-permuted to even-pairs-first
    order so the RoPE rotate-half pair swap becomes a contiguous
    32-partition block swap (plain DMAs; strided-partition DMA is broken).
  - v is produced in natural [s, e] layout (x as stationary operand), padded
    with a ones-column per head (65-wide slots) so the softmax denominator
    falls out of the same matmul that computes att@v.
  - Attention is computed as scoresT[sk, sq] = k_ropedT-stationary x
    q_ropedT, exp on ScalarE (no max subtraction: scores ~ N(0,1), exp is
    safe), then oT[dh, sq] = v_aug-stationary x E, which leaves oT in
    exactly the [d, s] layout the output projection needs as its stationary
    operand.
  - All matmuls run as float32r (TF32-like precision, ~10x better than
    bf16; measured end-to-end rel err 3.8e-4 vs the fp32 reference). The
    projections stream 256/288-wide moving chunks, which hit a fast PE
    streaming mode measured at ~25-60ns per 128x128 accumulation step.

Outputs of the 8 cores are gathered and reshaped on the host; b_out is added
on the host (it is all-zeros in the reference inputs anyway).
"""

import numpy as np
from contextlib import ExitStack

B, T, HH, WW, D = 2, 4, 24, 24, 1024
NH, HD = 16, 64
S = HH * WW            # 576
BT = B * T             # 8
NCORES = 8
P = 128
SQH = 288              # half of S; moving-dim per scores/att@v matmul
NKD = D // P           # 8 contraction tiles over D
S_TILES = [(0, 128), (128, 128), (256, 128), (384, 128), (512, 64)]
VSLOT = HD + 1         # 65: per-head v columns + ones column

_CACHE: dict = {}


def _rope_tables():
    """cos/sin tables in the permuted (evens-first) [128, S] block layout."""
    half = HD // 4     # 16
    inv = (1.0 / (10000.0 ** (np.arange(half, dtype=np.float32) / np.float32(half)))).astype(np.float32)
    th = np.arange(HH, dtype=np.float32)[:, None] * inv[None, :]          # [H, 16]
    tw = np.arange(WW, dtype=np.float32)[:, None] * inv[None, :]          # [W, 16]
    cosg = np.concatenate([
        np.broadcast_to(np.cos(th)[:, None, :], (HH, WW, half)),
        np.broadcast_to(np.cos(tw)[None, :, :], (HH, WW, half))], axis=-1).reshape(S, 2 * half)
    sing = np.concatenate([
        np.broadcast_to(np.sin(th)[:, None, :], (HH, WW, half)),
        np.broadcast_to(np.sin(tw)[None, :, :], (HH, WW, half))], axis=-1).reshape(S, 2 * half)
    cosb = np.concatenate([cosg, cosg], axis=1).T          # [64, S]
    sinb = np.concatenate([sing, -sing], axis=1).T         # [64, S] (pre-swapped)
    cosb = np.ascontiguousarray(np.vstack([cosb, cosb]).astype(np.float32))   # [128, S]
    sinb = np.ascontiguousarray(np.vstack([sinb, sinb]).astype(np.float32))
    return cosb, sinb


def _head_perm():
    """Permutation of w_qkv q/k rows: within each head, evens then odds."""
    perm64 = np.concatenate([np.arange(0, HD, 2), np.arange(1, HD, 2)])
    return (np.arange(NH)[:, None] * HD + perm64[None, :]).reshape(-1)     # [1024]


def _build_nc(repeat=1):
    import concourse.bacc as bacc
    import concourse.mybir as mybir
    from concourse.tile import TileContext

    f32 = mybir.dt.float32
    f32r = mybir.dt.float32r
    AF = mybir.ActivationFunctionType

    nc = bacc.Bacc("TRN2", target_bir_lowering=False, debug=False)
    xT_d = nc.dram_tensor("xT", [D, S], f32r, kind="ExternalInput").ap()
    wqk_d = nc.dram_tensor("wqkT", [D, 2 * D], f32r, kind="ExternalInput").ap()
    wv_d = nc.dram_tensor("wvT", [D, D], f32r, kind="ExternalInput").ap()
    wo_d = nc.dram_tensor("woT", [D, D], f32r, kind="ExternalInput").ap()
    cos_d = nc.dram_tensor("cosb", [P, S], f32, kind="ExternalInput").ap()
    sin_d = nc.dram_tensor("sinb", [P, S], f32, kind="ExternalInput").ap()
    ones_d = nc.dram_tensor("onesc", [P, 5 * NH], f32r, kind="ExternalInput").ap()
    out_d = nc.dram_tensor("out", [S, D], f32, kind="ExternalOutput").ap()

    with TileContext(nc) as tc, ExitStack() as ctx:
        const = ctx.enter_context(tc.tile_pool(name="const", bufs=1))
        wqkp = ctx.enter_context(tc.tile_pool(name="wqkp", bufs=3))
        wvp = ctx.enter_context(tc.tile_pool(name="wvp", bufs=3))
        wop = ctx.enter_context(tc.tile_pool(name="wop", bufs=3))
        rawp = ctx.enter_context(tc.tile_pool(name="rawp", bufs=3))
        m2p = ctx.enter_context(tc.tile_pool(name="m2p", bufs=2))
        ep = ctx.enter_context(tc.tile_pool(name="ep", bufs=12))
        r1p = ctx.enter_context(tc.tile_pool(name="r1p", bufs=4))
        rrp = ctx.enter_context(tc.tile_pool(name="rrp", bufs=4))
        stp = ctx.enter_context(tc.tile_pool(name="stp", bufs=3))
        psum = ctx.enter_context(tc.tile_pool(name="psum", bufs=8, space="PSUM"))

        # ---- resident tensors
        xt = const.tile([P, NKD * S], f32r, name="xt")
        cosb = const.tile([P, S], f32, name="cosb_t")
        sinb = const.tile([P, S], f32, name="sinb_t")
        roped = const.tile([P, 2 * NH * S], f32r, name="roped")    # 16 e-tiles (q then k)
        va = const.tile([P, 5 * NH * VSLOT], f32r, name="va")      # v, 65-wide head slots
        vav = va.rearrange("p (j h c) -> p j h c", j=5, c=VSLOT)
        oT = const.tile([P, NKD * S], f32r, name="oT")

        for _rep in range(repeat):
            # ---- q,k projection (+ RoPE) for one e-tile, pipelined per head-pair
            def emit_qk(et):
                ps0 = psum.tile([P, SQH], f32, tag="ps", name="ps_qk0")
                ps1 = psum.tile([P, SQH], f32, tag="ps", name="ps_qk1")
                wt = wqkp.tile([P, NKD * P], f32r, name="wt")
                nc.sync.dma_start(wt.rearrange("p (kt c) -> p kt c", c=P),
                                  wqk_d[:, et * P:(et + 1) * P].rearrange("(kt p) c -> p kt c", p=P))
                for kt in range(NKD):
                    w_r = wt[:, kt * P:(kt + 1) * P]
                    nc.tensor.matmul(ps0[:, :], w_r, xt[:, kt * S:kt * S + SQH],
                                     start=(kt == 0), stop=(kt == NKD - 1))
                    nc.tensor.matmul(ps1[:, :], w_r, xt[:, kt * S + SQH:kt * S + S],
                                     start=(kt == 0), stop=(kt == NKD - 1))
                raw = rawp.tile([P, S], f32, name="raw")
                nc.scalar.activation(raw[:, 0:SQH], ps0[:, :], AF.Copy)
                nc.scalar.activation(raw[:, SQH:S], ps1[:, :], AF.Copy)
                m2 = m2p.tile([P, S], f32, name="m2")
                for b0 in range(0, P, 64):
                    nc.vector.tensor_mul(m2[b0 + 32:b0 + 64, :], raw[b0:b0 + 32, :], sinb[b0:b0 + 32, :])
                    nc.gpsimd.tensor_mul(m2[b0:b0 + 32, :], raw[b0 + 32:b0 + 64, :], sinb[b0 + 32:b0 + 64, :])
                rsl = roped[:, et * S:(et + 1) * S]
                nc.vector.tensor_mul(rsl, raw[:, :], cosb[:, :])
                nc.vector.tensor_add(rsl, rsl, m2[:, :])

            # ---- attention for a head pair (both sq halves); the two heads sit on
            # disjoint PE row-groups (partitions 0:64 / 64:128), so interleaving
            # their scoresT matmuls lets them run concurrently in the array.
            def emit_att_pair(ti):
                qb = ti * S
                kb = (8 + ti) * S
                for hf in range(2):
                    col = slice(qb + hf * SQH, qb + (hf + 1) * SQH)
                    Es = {0: [], 1: []}
                    for j, (k0, kl) in enumerate(S_TILES):
                        pss = {}
                        for sub in range(2):
                            off = sub * 64
                            ps_s = psum.tile([P, SQH], f32, tag="ps", name="ps_s")
                            nc.tensor.matmul(ps_s[0:kl, :],
                                             roped[off:off + 64, kb + k0:kb + k0 + kl],
                                             roped[off:off + 64, col],
                                             start=True, stop=True)
                            pss[sub] = ps_s
                        for sub in range(2):
                            E = ep.tile([P, SQH], f32r, name="E")
                            nc.scalar.activation(E[0:kl, :], pss[sub][0:kl, :], AF.Exp, scale=0.125)
                            Es[sub].append(E)
                    for sub in range(2):
                        h = 2 * ti + sub
                        off = sub * 64
                        ps_o = psum.tile([P, SQH], f32, tag="ps", name="ps_o")
                        for j, (k0, kl) in enumerate(S_TILES):
                            nc.tensor.matmul(ps_o[0:VSLOT, :],
                                             vav[0:kl, j:j + 1, h:h + 1, :],
                                             Es[sub][j][0:kl, :],
                                             start=(j == 0), stop=(j == 4))
                        r1 = r1p.tile([1, SQH], f32, name="r1")
                        nc.vector.reciprocal(r1[:, :], ps_o[HD:HD + 1, :])
                        rr = rrp.tile([64, SQH], f32, name="rr")
                        nc.gpsimd.partition_broadcast(rr[:, :], r1[:, :])
                        nc.vector.tensor_mul(
                            oT[off:off + 64, ti * S + hf * SQH:ti * S + (hf + 1) * SQH],
                            ps_o[0:HD, :], rr[:, :])

            # ---- v projection first, then pair-pipelined qk+attention
            for nhf in range(2):
                for c in range(2):
                    psv = [psum.tile([P, 256], f32, tag="ps", name=f"ps_v{st}") for st in range(5)]
                    for kt2 in range(NKD // 2):
                        if nhf == 0 and c == 0:
                            for i in (2 * kt2, 2 * kt2 + 1):
                                nc.sync.dma_start(xt[:, i * S:(i + 1) * S], xT_d[i * P:(i + 1) * P, :])
                        wvt = wvp.tile([P, 512], f32r, name="wvt")
                        nc.sync.dma_start(wvt.rearrange("p (two cc) -> p two cc", cc=256),
                                          wv_d[kt2 * 2 * P:(kt2 * 2 + 2) * P,
                                               nhf * 512 + c * 256:nhf * 512 + (c + 1) * 256]
                                          .rearrange("(two p) cc -> p two cc", p=P))
                        for j in range(2):
                            kt = kt2 * 2 + j
                            for st, (s0, sl) in enumerate(S_TILES):
                                nc.tensor.matmul(psv[st][0:sl, :],
                                                 xt[:, kt * S + s0:kt * S + s0 + sl],
                                                 wvt[:, j * 256:(j + 1) * 256],
                                                 start=(kt == 0), stop=(kt == NKD - 1))
                    for st, (s0, sl) in enumerate(S_TILES):
                        dst = vav[0:sl, st:st + 1, nhf * 8 + c * 4:nhf * 8 + (c + 1) * 4, 0:HD]
                        vsrc = psv[st][0:sl, :].rearrange("p (h cc) -> p h cc", cc=HD)
                        nc.vector.tensor_copy(dst, vsrc)

            nc.sync.dma_start(cosb[:, :], cos_d[:, :])
            nc.sync.dma_start(sinb[:, :], sin_d[:, :])
            nc.sync.dma_start(va.rearrange("p (g c) -> p g c", c=VSLOT)[:, :, HD:HD + 1],
                              ones_d[:, :])
            for pr in range(8):
                emit_qk(pr)
                emit_qk(8 + pr)
            for ti in range(8):
                emit_att_pair(ti)

            # ---- output projection : out[s, e] = oT-tiles.T @ woT
            for nhf in range(2):
                for c in range(2):
                    pso = [psum.tile([P, 256], f32, tag="ps", name=f"ps_o{st}") for st in range(5)]
                    for kt2 in range(NKD // 2):
                        wot = wop.tile([P, 512], f32r, name="wot")
                        nc.sync.dma_start(wot.rearrange("p (two cc) -> p two cc", cc=256),
                                          wo_d[kt2 * 2 * P:(kt2 * 2 + 2) * P,
                                               nhf * 512 + c * 256:nhf * 512 + (c + 1) * 256]
                                          .rearrange("(two p) cc -> p two cc", p=P))
                        for j in range(2):
                            kt = kt2 * 2 + j
                            for st, (s0, sl) in enumerate(S_TILES):
                                nc.tensor.matmul(pso[st][0:sl, :],
                                                 oT[:, kt * S + s0:kt * S + s0 + sl],
                                                 wot[:, j * 256:(j + 1) * 256],
                                                 start=(kt == 0), stop=(kt == NKD - 1))
                    for st, (s0, sl) in enumerate(S_TILES):
                        stg = stp.tile([P, 256], f32, name="stg")
                        nc.vector.tensor_copy(stg[0:sl, :], pso[st][0:sl, :])
                        nc.sync.dma_start(out_d[s0:s0 + sl, nhf * 512 + c * 256:nhf * 512 + (c + 1) * 256],
                                          stg[0:sl, :])
    nc.compile()
    return nc


def _prep_inputs(x, w_qkv, w_out):
    x = np.asarray(x, dtype=np.float32)
    w_qkv = np.asarray(w_qkv, dtype=np.float32)
    w_out = np.asarray(w_out, dtype=np.float32)
    xr = x.reshape(BT, S, D)
    perm = _head_perm()
    wq = w_qkv[0:D][perm]
    wk = w_qkv[D:2 * D][perm]
    wqkT = np.ascontiguousarray(np.concatenate([wq, wk], axis=0).T)
    wvT = np.ascontiguousarray(w_qkv[2 * D:3 * D].T)
    woT = np.ascontiguousarray(w_out.T)
    cosb, sinb = _rope_tables()
    in_maps = []
    for i in range(NCORES):
        in_maps.append({
            "xT": np.ascontiguousarray(xr[i].T),
            "wqkT": wqkT, "wvT": wvT, "woT": woT,
            "cosb": cosb, "sinb": sinb,
            "onesc": np.ones((P, 5 * NH), dtype=np.float32),
        })
    return in_maps


def get_nc(repeat=1):
    key = f"nc{repeat}"
    if key not in _CACHE:
        _CACHE[key] = _build_nc(repeat)
    return _CACHE[key]


def kernel(x, w_qkv, w_out, b_out):
    from concourse import bass_utils
    nc = get_nc()
    in_maps = _prep_inputs(x, w_qkv, w_out)
    res = bass_utils.run_bass_kernel_spmd(nc, in_maps, core_ids=list(range(NCORES)))
    out = np.stack([res.results[i]["out"] for i in range(NCORES)], axis=0)
    out = out + np.asarray(b_out, dtype=np.float32)[None, None, :]
    return np.ascontiguousarray(out.reshape(B, T, HH, WW, D).astype(np.float32))



# revision 16
# speedup vs baseline: 1.4443x; 1.4443x over previous
"""Fused 2D-RoPE multi-head attention block for Trainium2, SPMD over 8 NeuronCores.

Problem: x[2,4,24,24,1024] -> qkv proj -> 16-head attention with 2-axis RoPE
-> out proj.  Data-parallel: the fused (b t) dim has 8 sequences; one
sequence (S=576 tokens, D=1024) per core.

v2 design (vs the f32r baseline):
  - All matmul operands are bf16 (f32 PSUM accumulation).  bf16 runs the PE
    at 1 cycle/moving-column for any moving width, halves weight/x DMA, and
    unlocks the DVE 2x perf mode for the RoPE elementwise ops.
  - One 5-bank PSUM tile ("big" tag) holds a full (head, sq-half) score
    block: 5 sk-tile matmuls write banks 0..4 at 512-f32 stride, then ONE
    big strided-AP Exp on the scalar engine produces the bf16 E tile.  This
    cuts Act-engine exp time from ~68us to ~44us per rep and exp instruction
    count 5x.  The same 5-bank tile is reused by the v / out projections
    (5 parallel 512-wide accumulation chains).
  - Software pipeline over 32 (head, sq-half) steps per rep: each step runs
    scores(i) + attv(i-1) + one qk projection chunk on the PE while exp(i)
    runs on Act and norm(i-1) on DVE/Pool.  PE never waits a full softmax.
  - Cross-rep overlap: xt / va / roped are double-buffered so rep r+1's
    v-projection and qk DMAs/matmuls overlap rep r's attention tail.
  - Engine balance per rep (model): PE ~101us, Act ~62us, DVE ~58us,
    Pool ~18us, DMA ~36us.

Outputs of the 8 cores are gathered and reshaped on the host; b_out is added
on the host (it is all-zeros in the reference inputs anyway).
"""

import numpy as np
from contextlib import ExitStack

B, T, HH, WW, D = 2, 4, 24, 24, 1024
NH, HD = 16, 64
S = HH * WW            # 576
BT = B * T             # 8
NCORES = 8
P = 128
SQH = 288              # sq half processed per scores/attv step
NKD = D // P           # 8 contraction tiles over D
S_TILES = [(0, 128), (128, 128), (256, 128), (384, 128), (512, 64)]
VSLOT = HD + 1         # 65: per-head v columns + ones column
BANK = 512             # f32 elements per PSUM bank

_CACHE: dict = {}


def _rope_tables():
    """cos/sin tables in the permuted (evens-first) [128, S] block layout."""
    half = HD // 4     # 16
    inv = (1.0 / (10000.0 ** (np.arange(half, dtype=np.float32) / np.float32(half)))).astype(np.float32)
    th = np.arange(HH, dtype=np.float32)[:, None] * inv[None, :]          # [H, 16]
    tw = np.arange(WW, dtype=np.float32)[:, None] * inv[None, :]          # [W, 16]
    cosg = np.concatenate([
        np.broadcast_to(np.cos(th)[:, None, :], (HH, WW, half)),
        np.broadcast_to(np.cos(tw)[None, :, :], (HH, WW, half))], axis=-1).reshape(S, 2 * half)
    sing = np.concatenate([
        np.broadcast_to(np.sin(th)[:, None, :], (HH, WW, half)),
        np.broadcast_to(np.sin(tw)[None, :, :], (HH, WW, half))], axis=-1).reshape(S, 2 * half)
    cosb = np.concatenate([cosg, cosg], axis=1).T          # [64, S]
    sinb = np.concatenate([sing, -sing], axis=1).T         # [64, S] (pre-swapped)
    cosb = np.ascontiguousarray(np.vstack([cosb, cosb]).astype(np.float32))   # [128, S]
    sinb = np.ascontiguousarray(np.vstack([sinb, sinb]).astype(np.float32))
    return cosb, sinb


def _head_perm():
    """Permutation of w_qkv q/k rows: within each head, evens then odds."""
    perm64 = np.concatenate([np.arange(0, HD, 2), np.arange(1, HD, 2)])
    return (np.arange(NH)[:, None] * HD + perm64[None, :]).reshape(-1)     # [1024]


def _build_nc(repeat=1):
    import concourse.bacc as bacc
    import concourse.mybir as mybir
    from concourse.tile import TileContext

    f32 = mybir.dt.float32
    bf16 = mybir.dt.bfloat16
    AF = mybir.ActivationFunctionType

    nc = bacc.Bacc("TRN2", target_bir_lowering=False, debug=False)
    xT_d = nc.dram_tensor("xT", [D, S], bf16, kind="ExternalInput").ap()
    wqk_d = nc.dram_tensor("wqkT", [D, 2 * D], bf16, kind="ExternalInput").ap()
    wv_d = nc.dram_tensor("wvT", [D, D], bf16, kind="ExternalInput").ap()
    wo_d = nc.dram_tensor("woT", [D, D], bf16, kind="ExternalInput").ap()
    cos_d = nc.dram_tensor("cosb", [P, S], bf16, kind="ExternalInput").ap()
    sin_d = nc.dram_tensor("sinb", [P, S], bf16, kind="ExternalInput").ap()
    out_d = nc.dram_tensor("out", [S, D], f32, kind="ExternalOutput").ap()

    with TileContext(nc) as tc, ExitStack() as ctx:
        const = ctx.enter_context(tc.tile_pool(name="const", bufs=1))
        wqkp = ctx.enter_context(tc.tile_pool(name="wqkp", bufs=5))
        wvp = ctx.enter_context(tc.tile_pool(name="wvp", bufs=10))
        wop = ctx.enter_context(tc.tile_pool(name="wop", bufs=10))
        rawp = ctx.enter_context(tc.tile_pool(name="rawp", bufs=2))
        m2p = ctx.enter_context(tc.tile_pool(name="m2p", bufs=2))
        ep = ctx.enter_context(tc.tile_pool(name="ep", bufs=3))
        r1p = ctx.enter_context(tc.tile_pool(name="r1p", bufs=2))
        rrp = ctx.enter_context(tc.tile_pool(name="rrp", bufs=2))
        stp = ctx.enter_context(tc.tile_pool(name="stp", bufs=2))
        ep2 = ctx.enter_context(tc.tile_pool(name="ep2", bufs=3))
        psum = ctx.enter_context(tc.tile_pool(name="psum", bufs=1, space="PSUM"))

        # ---- resident tensors (xt/va/roped double-buffered for cross-rep overlap)
        cosb = const.tile([P, S], bf16, name="cosb_t")
        sinb = const.tile([P, S], bf16, name="sinb_t")
        oT = const.tile([P, NKD * S], bf16, name="oT")
        xts = [const.tile([P, NKD * S], bf16, name=f"xt{i}") for i in range(2)]
        vas = [const.tile([P, 5 * NH * VSLOT], bf16, name=f"va{i}") for i in range(2)]
        ropeds = [const.tile([P, 2 * NH * S], bf16, name=f"roped{i}") for i in range(2)]
        # block-diagonal stationary staging for the paired leftover-sk scores
        # matmul (two per-pair tiles; off-diagonal blocks stay zero forever)
        bds = [const.tile([P, P], bf16, name=f"bd{i}") for i in range(2)]

        nc.sync.dma_start(cosb[:, :], cos_d[:, :])
        nc.sync.dma_start(sinb[:, :], sin_d[:, :])
        # ones columns for the softmax denominator: set whole va to 1.0 once;
        # v copies overwrite cols 0:64 of each head slot, col 64 stays 1.0.
        for va in vas:
            nc.vector.memset(va[:, :], 1.0)
        for bd in bds:
            nc.gpsimd.memset(bd[:, :], 0.0)

        for _rep in range(repeat):
            bx = _rep % 2
            xt, va, roped = xts[bx], vas[bx], ropeds[bx]

            # ---- x for this rep (consumed by v-proj and qk-proj)
            for kt in range(NKD):
                nc.sync.dma_start(xt[:, kt * S:(kt + 1) * S], xT_d[kt * P:(kt + 1) * P, :])

            # ---- v projection: per half, 5 sequential 512-wide chains (st
            # outer) over a ring of persistent weight chunks, so each chain's
            # PSUM->va copy overlaps the next chain's matmuls.
            for half in range(2):
                wvts = []
                for kt in range(NKD):
                    wvt = wvp.tile([P, BANK], bf16, name="wvt")
                    nc.sync.dma_start(wvt[:, :],
                                      wv_d[kt * P:(kt + 1) * P, half * BANK:(half + 1) * BANK])
                    wvts.append(wvt)
                vav = va.rearrange("p (j h c) -> p j h c", j=5, c=VSLOT)
                for st, (s0, sl) in enumerate(S_TILES):
                    tv = psum.tile([P, BANK], f32, tag="sm", bufs=4, name="tv")
                    for kt in range(NKD):
                        nc.tensor.matmul(tv[0:sl, :],
                                         xt[:, kt * S + s0:kt * S + s0 + sl],
                                         wvts[kt][:, :],
                                         start=(kt == 0), stop=(kt == NKD - 1))
                    nc.scalar.activation(
                        vav[0:sl, st:st + 1, half * 8:(half + 1) * 8, 0:HD],
                        tv[0:sl, :].rearrange("p (h c) -> p h c", c=HD),
                        AF.Copy)
                    if st == 4:
                        # replicate leftover-sk v rows to partitions 64:128 so
                        # odd heads' paired-leftover attv matmul (whose E rows
                        # sit at partitions 64:128) has a matching stationary
                        nc.scalar.activation(
                            vav[64:64 + sl, st:st + 1, half * 8:(half + 1) * 8, 0:HD],
                            tv[0:sl, :].rearrange("p (h c) -> p h c", c=HD),
                            AF.Copy)

            # ---- qk projection chunks (each chunk = one sq half of one e-tile)
            def emit_qk_chunk(et, chunk, wt):
                ps = psum.tile([P, SQH], f32, tag="sm", bufs=4, name="ps_qk")
                c0 = chunk * SQH
                for kt in range(NKD):
                    nc.tensor.matmul(ps[:, :], wt[:, kt * P:(kt + 1) * P],
                                     xt[:, kt * S + c0:kt * S + c0 + SQH],
                                     start=(kt == 0), stop=(kt == NKD - 1))
                return ps

            def emit_qk_rope(et, raw, chunk):
                # RoPE for one sq-half, right after its PSUM->raw copy: keeps
                # roped ready well before the next pair's scores matmuls.
                c0, c1 = chunk * SQH, (chunk + 1) * SQH
                m2 = m2p.tile([P, SQH], bf16, name="m2")
                for b0 in range(0, P, 64):
                    nc.vector.tensor_mul(m2[b0 + 32:b0 + 64, :], raw[b0:b0 + 32, c0:c1], sinb[b0:b0 + 32, c0:c1])
                    nc.vector.tensor_mul(m2[b0:b0 + 32, :], raw[b0 + 32:b0 + 64, c0:c1], sinb[b0 + 32:b0 + 64, c0:c1])
                rsl = roped[:, et * S + c0:et * S + c1]
                nc.vector.tensor_mul(rsl, raw[:, c0:c1], cosb[:, c0:c1])
                nc.vector.tensor_add(rsl, rsl, m2[:, :])

            # qk pipeline: weights DMA'd two pairs ahead of their chunks
            qk_wt = {}
            qk_raw = {}

            def emit_qk_dma(et):
                wt = wqkp.tile([P, NKD * P], bf16, name="wt")
                nc.sync.dma_start(wt.rearrange("p (kt c) -> p kt c", c=P),
                                  wqk_d[:, et * P:(et + 1) * P].rearrange("(kt p) c -> p kt c", p=P))
                qk_wt[et] = wt

            def emit_qk_step(et, sub_step):
                if sub_step == 0:
                    qk_raw[et] = rawp.tile([P, S], bf16, name="raw")
                    wt = qk_wt[et]
                else:
                    wt = qk_wt.pop(et)
                raw = qk_raw[et] if sub_step == 0 else qk_raw.pop(et)
                ps = emit_qk_chunk(et, sub_step, wt)
                nc.scalar.activation(raw[:, sub_step * SQH:(sub_step + 1) * SQH], ps[:, :], AF.Copy)
                emit_qk_rope(et, raw, sub_step)

            # ---- attention step pieces
            def emit_bd_fill(p):
                # stage the two heads' leftover-sk (512:576) k tiles into the
                # block-diagonal stationary (zeros elsewhere, set once)
                kb = (8 + p) * S
                bd = bds[p % 2]
                nc.vector.tensor_copy(bd[0:64, 0:64], roped[0:64, kb + 512:kb + 576])
                nc.vector.tensor_copy(bd[64:128, 64:128], roped[64:128, kb + 512:kb + 576])

            EL_cur = {}

            def emit_scores_exp(p, sub, hf):
                off = sub * 64
                kb = (8 + p) * S
                mv = roped[off:off + 64, p * S + hf * SQH:p * S + (hf + 1) * SQH]
                t4 = psum.tile([P, 4 * BANK], f32, tag="big", name="t4")
                for j in range(4):
                    k0 = j * P
                    nc.tensor.matmul(t4[0:P, j * BANK:j * BANK + SQH],
                                     roped[off:off + 64, kb + k0:kb + k0 + P],
                                     mv, start=True, stop=True)
                if sub == 0:
                    # paired leftover sk tile for BOTH heads via the
                    # block-diagonal stationary: rows 0:64 -> head A scores,
                    # rows 64:128 -> head B scores
                    psL = psum.tile([P, SQH], f32, tag="sm", bufs=4, name="psL")
                    nc.tensor.matmul(psL[:, :], bds[p % 2][:, :],
                                     roped[:, p * S + hf * SQH:p * S + (hf + 1) * SQH],
                                     start=True, stop=True)
                    EL = ep2.tile([P, SQH], bf16, name="EL")
                    nc.scalar.activation(EL[:, :], psL[:, :], AF.Exp, scale=0.125)
                    EL_cur[hf] = EL
                E = ep.tile([P, 4 * SQH], bf16, name="E")
                nc.scalar.activation(
                    E.rearrange("p (j c) -> p j c", c=SQH),
                    t4.rearrange("p (j c) -> p j c", c=BANK)[:, :, 0:SQH],
                    AF.Exp, scale=0.125)
                return E, EL_cur[hf]

            def emit_attv_norm(p, sub, hf, E, EL):
                off = sub * 64
                h = 2 * p + sub
                vav = va.rearrange("p (j h c) -> p j h c", j=5, c=VSLOT)
                ps_o = psum.tile([P, SQH], f32, tag="sm", bufs=4, name="ps_o")
                for j in range(4):
                    nc.tensor.matmul(ps_o[0:VSLOT, :],
                                     vav[0:P, j:j + 1, h:h + 1, :],
                                     E[0:P, j * SQH:(j + 1) * SQH],
                                     start=(j == 0), stop=False)
                nc.tensor.matmul(ps_o[0:VSLOT, :],
                                 vav[off:off + 64, 4:5, h:h + 1, :],
                                 EL[off:off + 64, :],
                                 start=False, stop=True)
                r1 = r1p.tile([1, SQH], f32, name="r1")
                nc.vector.reciprocal(r1[:, :], ps_o[HD:HD + 1, :])
                rr = rrp.tile([64, SQH], f32, name="rr")
                nc.gpsimd.partition_broadcast(rr[:, :], r1[:, :])
                nc.vector.tensor_mul(
                    oT[off:off + 64, p * S + hf * SQH:p * S + (hf + 1) * SQH],
                    ps_o[0:HD, :], rr[:, :])

            # ---- pipelined pair loop: 32 (p, sub, hf) half-steps
            for et in (8, 0, 9, 1):
                emit_qk_dma(et)
            for et in (8, 0):
                emit_qk_step(et, 0)
                emit_qk_step(et, 1)
            emit_bd_fill(0)
            prev = None
            for p in range(8):
                for sub in range(2):
                    for hf in range(2):
                        if p < 6 and hf == 0:
                            # weights for pair p+2, two pairs ahead of use
                            emit_qk_dma((8 + p + 2) if sub == 0 else (p + 2))
                        if p < 7:
                            # chunks for pair p+1: k e-tile first (scores need
                            # the full k e-tile roped), then q
                            et = (8 + p + 1) if sub == 0 else (p + 1)
                            emit_qk_step(et, hf)
                        if p < 7 and sub == 1 and hf == 0:
                            emit_bd_fill(p + 1)
                        E, EL = emit_scores_exp(p, sub, hf)
                        if prev is not None:
                            emit_attv_norm(*prev)
                        prev = (p, sub, hf, E, EL)
            emit_attv_norm(*prev)

            # ---- output projection: out[s, e] = oT-tiles.T @ woT (st-outer,
            # staging copy + store of chain st overlap chain st+1)
            for half in range(2):
                wots = []
                for kt in range(NKD):
                    wot = wop.tile([P, BANK], bf16, name="wot")
                    nc.sync.dma_start(wot[:, :],
                                      wo_d[kt * P:(kt + 1) * P, half * BANK:(half + 1) * BANK])
                    wots.append(wot)
                for st, (s0, sl) in enumerate(S_TILES):
                    to = psum.tile([P, BANK], f32, tag="sm", bufs=4, name="to")
                    for kt in range(NKD):
                        nc.tensor.matmul(to[0:sl, :],
                                         oT[:, kt * S + s0:kt * S + s0 + sl],
                                         wots[kt][:, :],
                                         start=(kt == 0), stop=(kt == NKD - 1))
                    stg = stp.tile([P, BANK], f32, name="stg")
                    nc.scalar.activation(stg[0:sl, :], to[0:sl, :], AF.Copy)
                    nc.gpsimd.dma_start(out_d[s0:s0 + sl, half * BANK:(half + 1) * BANK],
                                        stg[0:sl, :])
    nc.compile()
    return nc


def _prep_inputs(x, w_qkv, w_out):
    import ml_dtypes
    bf = ml_dtypes.bfloat16
    x = np.asarray(x, dtype=np.float32)
    w_qkv = np.asarray(w_qkv, dtype=np.float32)
    w_out = np.asarray(w_out, dtype=np.float32)
    xr = x.reshape(BT, S, D)
    perm = _head_perm()
    wq = w_qkv[0:D][perm]
    wk = w_qkv[D:2 * D][perm]
    wqkT = np.ascontiguousarray(np.concatenate([wq, wk], axis=0).T.astype(bf))
    wvT = np.ascontiguousarray(w_qkv[2 * D:3 * D].T.astype(bf))
    woT = np.ascontiguousarray(w_out.T.astype(bf))
    cosb, sinb = _rope_tables()
    in_maps = []
    for i in range(NCORES):
        in_maps.append({
            "xT": np.ascontiguousarray(xr[i].T.astype(bf)),
            "wqkT": wqkT, "wvT": wvT, "woT": woT,
            "cosb": cosb.astype(bf), "sinb": sinb.astype(bf),
        })
    return in_maps


def get_nc(repeat=1):
    key = f"nc{repeat}"
    if key not in _CACHE:
        _CACHE[key] = _build_nc(repeat)
    return _CACHE[key]


def kernel(x, w_qkv, w_out, b_out):
    from concourse import bass_utils
    nc = get_nc()
    in_maps = _prep_inputs(x, w_qkv, w_out)
    res = bass_utils.run_bass_kernel_spmd(nc, in_maps, core_ids=list(range(NCORES)))
    out = np.stack([res.results[i]["out"] for i in range(NCORES)], axis=0)
    out = out + np.asarray(b_out, dtype=np.float32)[None, None, :]
    return np.ascontiguousarray(out.reshape(B, T, HH, WW, D).astype(np.float32))
